# revision 20
# baseline (speedup 1.0000x reference)
"""GAT-style attention kernel for Trainium2, data-parallel over batch on 8 cores.

Math: the reference computes
    e[i,j]  = lr_row[i] + lr_col[j]            (rank-1 score structure)
    atten   = softmax_j(where(mask>0, e, -1e9))
    out     = atten @ (x @ Wx.T + bx)
lr_row[i] is constant along the softmax axis j, so it cancels:
    atten[i,j] = mask[i,j] * w[j] / sum_j mask[i,j] * w[j],
    w[j] = exp(lr_col[j] - max_j lr_col[j])
and because attention rows sum to 1 the bias rides inside the numerator:
    out = (M @ (w * (xv0 + bx))) / (M @ w),   xv0 = x @ Wx.T
So the kernel is one [N,N] x [N,129] matmul per batch, normalized row-wise.

Host-side prep (pure layout; no model math): the 0/1 mask is cast to bf16
(exact) and laid out transposed + tiled so each output row-strip's lhsT
chunks are contiguous 4KB-per-partition DMA lines:
    L[ti, p, tj, i] = mask[ti*128+i, tj*128+p]
x is likewise uploaded pre-transposed in bf16.  This halves mask HBM
traffic (8MB/core) and removes all on-device casts/transposes.

Per core (batch b):
  - setup: xv/col projections from xT, lr_col -> global max -> w,
    U[:,tj] = [w*(xv0+bx) | w] in bf16
  - main loop over 16 row strips: one 512KB strip DMA (sync queue,
    8-deep rotation) + 16 accumulating matmuls into psum [128,132]
  - finish (skewed 3 strips back): reciprocal of psum col 128 (DVE),
    ACT scale-copy psum->SBUF f32, store on scalar queue
"""

import os
import sys

import numpy as np

for _p in ("/opt/trn_rl_repo",):
    if _p not in sys.path and os.path.isdir(_p):
        sys.path.append(_p)

import concourse.bacc as bacc
import concourse.bass as bass
import concourse.bass_isa as bass_isa
import concourse.tile as tile
from concourse import mybir
from concourse.bass_utils import run_bass_kernel_spmd

B, N, DIN, DOUT, DA = 8, 2048, 128, 128, 2
NEG_SLOPE = 0.2
P = 128
UC = 132  # U free width: 128 numerator cols + 1 denom col + 3 pad

F32 = mybir.dt.float32
BF16 = mybir.dt.bfloat16
F8 = mybir.dt.float8e4
MASK_FP8 = True
MDT = F8 if MASK_FP8 else BF16


def build(n=N, skew=3, load_groups=(4, 4, 4, 2, 1, 1)):
    """Build the single-core program (all 8 cores run it SPMD)."""
    nt = n // P
    assert sum(load_groups) == nt
    nc = bacc.Bacc(
        "TRN2",
        target_bir_lowering=False,
        debug=False,
        enable_asserts=False,
        num_devices=1,
    )
    xT_d = nc.dram_tensor("xT", [DIN, n], BF16, kind="ExternalInput").ap()
    # maskt rows are the tiled-transposed layout documented above
    m_d = nc.dram_tensor("maskt", [n, n], MDT, kind="ExternalInput").ap()
    # wcomb = [Wx.T | Wc.T]  (precomputed on host; tiny params)
    wcomb_d = nc.dram_tensor("wcomb", [DIN, DOUT + DA], BF16, kind="ExternalInput").ap()
    # pk = [a2 broadcast | bx broadcast]  f32 [P, DA+DOUT]
    pk_d = nc.dram_tensor("pk", [P, DA + DOUT], F32, kind="ExternalInput").ap()
    out_d = nc.dram_tensor("out", [n, DOUT], F32, kind="ExternalOutput").ap()

    from contextlib import ExitStack

    with tile.TileContext(nc) as tc, ExitStack() as ctx:
        consts = ctx.enter_context(tc.tile_pool(name="consts", bufs=1))
        small = ctx.enter_context(tc.tile_pool(name="small", bufs=2))
        mpool = ctx.enter_context(tc.tile_pool(name="mpool", bufs=1))
        opool = ctx.enter_context(tc.tile_pool(name="opool", bufs=8))
        # single 8-bank PSUM ring: 8 projection tiles first, then the main
        # accumulators recycle the same banks (WAR resolves via the U-scales
        # which consume the projection results directly from PSUM)
        ps = ctx.enter_context(tc.tile_pool(name="ps", bufs=8, space="PSUM"))

        # ---- sync queue: xT halves first, then 8 x 1MB mask double-strips;
        # scalar queue: 2 packed const loads, later the output stores ----
        wcomb = consts.tile([DIN, DOUT + DA], BF16)
        nc.scalar.dma_start(wcomb[:], wcomb_d)
        pk = consts.tile([P, DA + DOUT], F32)
        nc.scalar.dma_start(pk[:], pk_d)
        a2b = pk[:, 0:DA]
        bxb = pk[:, DA : DA + DOUT]
        xTs = consts.tile([DIN, n], BF16)
        for q in range(2):
            h = n // 2
            nc.sync.dma_start(xTs[:, q * h : (q + 1) * h], xT_d[:, q * h : (q + 1) * h])

        # one load per group; group sizes descend so the tail is fine-grained
        # and total load-DMA count stays <= the 8 hw sem lanes (no lane reuse
        # for loads => prefetch never gates on PE consumption). All strips
        # live in one persistent SBUF region (64KB/partition).
        mall = consts.tile([P, nt, n], MDT)
        base = 0
        for gsz in load_groups:
            if gsz == 1:
                nc.sync.dma_start(mall[:, base], m_d[base * P : (base + 1) * P, :])
            else:
                nc.sync.dma_start(
                    mall[:, base : base + gsz],
                    m_d[base * P : (base + gsz) * P, :].rearrange(
                        "(s p) f -> p s f", p=P
                    ),
                )
            base += gsz

        # U pad columns only (cols DOUT+1..UC never written otherwise)
        U = consts.tile([P, nt, UC], BF16)
        nc.vector.memset(U[:, :, DOUT + 1 : UC], 0)

        # ---- HAM warmup: ~3.5us of tiny matmuls during the initial DMA
        # latency window so projections and early strips run at 2.4GHz ----
        warm_in = small.tile([P, 1], BF16)
        nc.vector.memset(warm_in[:], 0)
        warm_ps = ps.tile([P, 1], F32, tag="ring", name="warm_ps")
        for _ in range(40):
            nc.tensor.matmul(
                warm_ps[0:1, 0:1], warm_in[:], warm_in[:], start=True, stop=True
            )

        # ---- projections: pxv[j,130] = xT_chunk.T @ [WxT | WcT], pairs per
        # psum tile; results stay in PSUM until the U-scales consume them.
        # Only the tiny lr_col columns get copied out for the w chain. ----
        colbuf = consts.tile([P, nt, DA], F32)
        PW = DOUT + DA
        pxvs = []
        for g in range(nt // 2):
            # flat [P, 2*PW] so the tile fits one 2KB psum bank
            pxv = ps.tile([P, 2 * PW], F32, tag="ring", name=f"pxv{g}")
            pxvs.append(pxv)
            for s in range(2):
                t = 2 * g + s
                nc.tensor.matmul(
                    pxv[:, s * PW : s * PW + PW],
                    xTs[:, t * P : (t + 1) * P], wcomb[:],
                    start=True, stop=True,
                )
            nc.vector.tensor_copy(
                colbuf[:, 2 * g : 2 * g + 2],
                pxv[:].rearrange("p (s c) -> p s c", s=2)[:, :, DOUT : DOUT + DA],
            )

        # ---- lr_col -> w = exp(lrc); the softmax max-shift cancels in
        # Num/denom and |lrc| is O(1), so no global max needed ----
        colp = colbuf[:]  # [P, nt, 2] contiguous
        c02 = small.tile([P, nt, DA], F32)
        nc.vector.tensor_scalar_mul(c02[:], colp, NEG_SLOPE)
        clr = small.tile([P, nt, DA], F32)
        nc.vector.tensor_max(clr[:], colp, c02[:])
        lr0 = small.tile([P, nt], F32)
        nc.vector.tensor_scalar(
            lr0[:], clr[:, :, 0], a2b[:, 0:1], None, mybir.AluOpType.mult
        )
        lr1 = small.tile([P, nt], F32)
        nc.vector.tensor_scalar(
            lr1[:], clr[:, :, 1], a2b[:, 1:2], None, mybir.AluOpType.mult
        )
        lrc = small.tile([P, nt], F32)
        nc.vector.tensor_add(lrc[:], lr0[:], lr1[:])
        w_all = consts.tile([P, nt], F32)
        nc.scalar.activation(w_all[:], lrc[:], mybir.ActivationFunctionType.Exp)

        # ---- U chunks [P, nt, UC] bf16: U[:,:,0:128]=w*xv0, U[:,:,128]=w ----
        # w column first so main MMs gate only on their own chunk's scale
        nc.vector.tensor_copy(U[:, :, DOUT], w_all[:])
        for t in range(nt):
            src_v = pxvs[t // 2][:, (t % 2) * PW : (t % 2) * PW + DOUT]
            if t % 2 == 0:
                nc.vector.tensor_scalar(
                    U[:, t, 0:DOUT], src_v, w_all[:, t : t + 1], None,
                    mybir.AluOpType.mult,
                )
            else:
                nc.scalar.activation(
                    U[:, t, 0:DOUT], src_v,
                    mybir.ActivationFunctionType.Copy, scale=w_all[:, t : t + 1],
                )

        # ---- main loop over output row strips ----
        paccs = [None] * nt

        def mm(ti, tj):
            if paccs[ti] is None:
                paccs[ti] = ps.tile([P, UC], F32, tag="ring", name=f"pacc{ti}")
            nc.tensor.matmul(
                paccs[ti][:, 0 : DOUT + 1],
                mall[:, ti, tj * P : (tj + 1) * P],
                U[:, tj, 0 : DOUT + 1],
                start=(tj == 0),
                stop=(tj == nt - 1),
            )

        def finish(k):
            rec = small.tile([P, 1], F32, tag="rec")
            nc.vector.reciprocal(rec[:], paccs[k][:, DOUT : DOUT + 1])
            o1 = opool.tile([P, DOUT], F32, tag="o1")
            nc.vector.tensor_scalar(
                o1[:], paccs[k][:, 0:DOUT], rec[:], None, mybir.AluOpType.mult
            )
            o2 = opool.tile([P, DOUT], F32, tag="o2")
            nc.vector.tensor_add(o2[:], o1[:], bxb[:])
            # stores ride the SWDGE lanes so they never steal an HWDGE sem
            # lane from a mask load; the last two use HWDGE (lower latency on
            # the kernel tail, recycling lanes whose consumers are long done)
            if k >= nt - 2:
                nc.scalar.dma_start(out_d[k * P : (k + 1) * P, :], o2[:])
            else:
                nc.gpsimd.dma_start(out_d[k * P : (k + 1) * P, :], o2[:])

        for ti in range(nt):
            for tj in range(nt):
                mm(ti, tj)
            if ti >= skew:
                finish(ti - skew)
        for ti in range(max(0, nt - skew), nt):
            finish(ti)

    nc.compile()
    return nc


def host_inputs(xb_bf, L_b, wc, pk):
    """Per-core input map for batch b (weights replicated, host-prepped)."""
    return {
        "xT": np.ascontiguousarray(xb_bf.T),
        "maskt": L_b,
        "wcomb": wc,
        "pk": pk,
    }


_cached = {}


def _get_nc():
    if "nc" not in _cached:
        _cached["nc"] = build()
    return _cached["nc"]


def _install_ntff_shim():
    """The agent image's antenv lacks axon_hooks; synthesize it so
    run_bass_kernel_spmd(trace=True) can reach the .so's NTFF profiler."""
    import types

    try:
        import antenv.axon_hooks  # noqa: F401

        return True
    except ImportError:
        pass
    try:
        import antenv
        from trn_agent_boot.trn_boot import _ntff_profile_via_ctypes

        hook = _ntff_profile_via_ctypes("/opt/axon/libaxon_pjrt.so")
        mod = types.ModuleType("antenv.axon_hooks")
        _state = {"hook": hook}
        mod.set_axon_ntff_profile_hook = lambda h: _state.__setitem__("hook", h)
        mod.get_axon_ntff_profile_hook = lambda: _state["hook"]
        sys.modules["antenv.axon_hooks"] = mod
        antenv.axon_hooks = mod
        return hook is not None
    except Exception as e:
        print(f"ntff shim failed: {e}", file=sys.stderr)
        return False


def kernel(x, mask, Wr, Wc, Wcat, Wx, bx, _trace=False, **_unused):
    import ml_dtypes

    BF = ml_dtypes.bfloat16
    x = np.asarray(x)
    mask = np.asarray(mask)
    Wc = np.asarray(Wc)
    Wcat = np.asarray(Wcat)
    Wx = np.asarray(Wx)
    bx = np.asarray(bx)
    nc = _get_nc()
    if _trace:
        _trace = _install_ntff_shim()

    nt = N // P
    xb = x.astype(BF)  # [B, N, DIN]
    # tiled transpose: L[b, ti, p, tj, i] = mask[b, ti*128+i, tj*128+p]
    MD = ml_dtypes.float8_e4m3fn if MASK_FP8 else BF
    mb = mask.astype(np.float32).astype(MD)  # 0/1 exact in bf16/fp8
    L = np.ascontiguousarray(
        mb.reshape(B, nt, P, nt, P).transpose(0, 1, 4, 3, 2)
    ).reshape(B, N, N)
    wc = np.ascontiguousarray(np.concatenate([Wx.T, Wc.T], axis=1).astype(BF))
    pk1 = np.concatenate([Wcat[DA:].reshape(1, DA), bx.reshape(1, DOUT)], axis=1)
    pk = np.ascontiguousarray(
        np.broadcast_to(pk1, (P, DA + DOUT)), dtype=np.float32
    )
    in_maps = [host_inputs(xb[b], L[b], wc, pk) for b in range(B)]
    res = run_bass_kernel_spmd(nc, in_maps, core_ids=list(range(B)), trace=_trace)
    out = np.stack([res.results[c]["out"] for c in range(B)]).astype(np.float32)
    if _trace:
        kernel.last_results = res
    return out


# revision 21
# speedup vs baseline: 1.0308x; 1.0308x over previous
"""GAT-style attention kernel for Trainium2, data-parallel over batch on 8 cores.

Math: the reference computes
    e[i,j]  = lr_row[i] + lr_col[j]            (rank-1 score structure)
    atten   = softmax_j(where(mask>0, e, -1e9))
    out     = atten @ (x @ Wx.T + bx)
lr_row[i] is constant along the softmax axis j, so it cancels:
    atten[i,j] = mask[i,j] * w[j] / sum_j mask[i,j] * w[j],
    w[j] = exp(lr_col[j] - max_j lr_col[j])
and because attention rows sum to 1 the bias rides inside the numerator:
    out = (M @ (w * (xv0 + bx))) / (M @ w),   xv0 = x @ Wx.T
So the kernel is one [N,N] x [N,129] matmul per batch, normalized row-wise.

Host-side prep (pure layout; no model math): the 0/1 mask is cast to bf16
(exact) and laid out transposed + tiled so each output row-strip's lhsT
chunks are contiguous 4KB-per-partition DMA lines:
    L[ti, p, tj, i] = mask[ti*128+i, tj*128+p]
x is likewise uploaded pre-transposed in bf16.  This halves mask HBM
traffic (8MB/core) and removes all on-device casts/transposes.

Per core (batch b):
  - setup: xv/col projections from xT, lr_col -> global max -> w,
    U[:,tj] = [w*(xv0+bx) | w] in bf16
  - main loop over 16 row strips: one 512KB strip DMA (sync queue,
    8-deep rotation) + 16 accumulating matmuls into psum [128,132]
  - finish (skewed 3 strips back): reciprocal of psum col 128 (DVE),
    ACT scale-copy psum->SBUF f32, store on scalar queue
"""

import os
import sys

import numpy as np

for _p in ("/opt/trn_rl_repo",):
    if _p not in sys.path and os.path.isdir(_p):
        sys.path.append(_p)

import concourse.bacc as bacc
import concourse.bass as bass
import concourse.bass_isa as bass_isa
import concourse.tile as tile
from concourse import mybir
from concourse.bass_utils import run_bass_kernel_spmd

B, N, DIN, DOUT, DA = 8, 2048, 128, 128, 2
NEG_SLOPE = 0.2
P = 128
UC = 132  # U free width: 128 numerator cols + 1 denom col + 3 pad

F32 = mybir.dt.float32
BF16 = mybir.dt.bfloat16
F8 = mybir.dt.float8e4
MASK_FP8 = True
MDT = F8 if MASK_FP8 else BF16


def build(n=N, skew=3, load_groups=(1, 3, 4, 4, 2, 1, 1)):
    """Build the single-core program (all 8 cores run it SPMD)."""
    nt = n // P
    assert sum(load_groups) == nt
    nc = bacc.Bacc(
        "TRN2",
        target_bir_lowering=False,
        debug=False,
        enable_asserts=False,
        num_devices=1,
    )
    xT_d = nc.dram_tensor("xT", [DIN, n], BF16, kind="ExternalInput").ap()
    # maskt rows are the tiled-transposed layout documented above
    m_d = nc.dram_tensor("maskt", [n, n], MDT, kind="ExternalInput").ap()
    # wcomb = [Wx.T | Wc.T]  (precomputed on host; tiny params)
    wcomb_d = nc.dram_tensor("wcomb", [DIN, DOUT + DA], BF16, kind="ExternalInput").ap()
    # pk = [a2 broadcast | bx broadcast]  f32 [P, DA+DOUT]
    pk_d = nc.dram_tensor("pk", [P, DA + DOUT], F32, kind="ExternalInput").ap()
    out_d = nc.dram_tensor("out", [n, DOUT], F32, kind="ExternalOutput").ap()

    from contextlib import ExitStack

    with tile.TileContext(nc) as tc, ExitStack() as ctx:
        consts = ctx.enter_context(tc.tile_pool(name="consts", bufs=1))
        small = ctx.enter_context(tc.tile_pool(name="small", bufs=2))
        mpool = ctx.enter_context(tc.tile_pool(name="mpool", bufs=1))
        opool = ctx.enter_context(tc.tile_pool(name="opool", bufs=8))
        # single 8-bank PSUM ring: 8 projection tiles first, then the main
        # accumulators recycle the same banks (WAR resolves via the U-scales
        # which consume the projection results directly from PSUM)
        ps = ctx.enter_context(tc.tile_pool(name="ps", bufs=8, space="PSUM"))

        # ---- sync queue: xT halves first, then 8 x 1MB mask double-strips;
        # scalar queue: 2 packed const loads, later the output stores ----
        wcomb = consts.tile([DIN, DOUT + DA], BF16)
        nc.scalar.dma_start(wcomb[:], wcomb_d)
        pk = consts.tile([P, DA + DOUT], F32)
        nc.scalar.dma_start(pk[:], pk_d)
        a2b = pk[:, 0:DA]
        bxb = pk[:, DA : DA + DOUT]
        xTs = consts.tile([DIN, n], BF16)
        for q in range(2):
            h = n // 2
            nc.sync.dma_start(xTs[:, q * h : (q + 1) * h], xT_d[:, q * h : (q + 1) * h])

        # one load per group; group sizes descend so the tail is fine-grained
        # and total load-DMA count stays <= the 8 hw sem lanes (no lane reuse
        # for loads => prefetch never gates on PE consumption). All strips
        # live in one persistent SBUF region (64KB/partition).
        mall = consts.tile([P, nt, n], MDT)
        base = 0
        for gsz in load_groups:
            if gsz == 1:
                nc.sync.dma_start(mall[:, base], m_d[base * P : (base + 1) * P, :])
            else:
                nc.sync.dma_start(
                    mall[:, base : base + gsz],
                    m_d[base * P : (base + gsz) * P, :].rearrange(
                        "(s p) f -> p s f", p=P
                    ),
                )
            base += gsz

        # U pad columns only (cols DOUT+1..UC never written otherwise)
        U = consts.tile([P, nt, UC], BF16)
        nc.vector.memset(U[:, :, DOUT + 1 : UC], 0)

        # ---- HAM warmup: ~3.5us of tiny matmuls during the initial DMA
        # latency window so projections and early strips run at 2.4GHz ----
        warm_in = small.tile([P, 64], BF16)
        nc.vector.memset(warm_in[:], 0)
        warm_ps = ps.tile([P, 64], F32, tag="ring", name="warm_ps")
        for _ in range(48):
            nc.tensor.matmul(
                warm_ps[0:1, :], warm_in[:, 0:1], warm_in[:], start=True, stop=True
            )

        # ---- projections: pxv[j,130] = xT_chunk.T @ [WxT | WcT], pairs per
        # psum tile; results stay in PSUM until the U-scales consume them.
        # Only the tiny lr_col columns get copied out for the w chain. ----
        colbuf = consts.tile([P, nt, DA], F32)
        PW = DOUT + DA
        pxvs = []
        for g in range(nt // 2):
            # flat [P, 2*PW] so the tile fits one 2KB psum bank
            pxv = ps.tile([P, 2 * PW], F32, tag="ring", name=f"pxv{g}")
            pxvs.append(pxv)
            for s in range(2):
                t = 2 * g + s
                nc.tensor.matmul(
                    pxv[:, s * PW : s * PW + PW],
                    xTs[:, t * P : (t + 1) * P], wcomb[:],
                    start=True, stop=True,
                )
            nc.vector.tensor_copy(
                colbuf[:, 2 * g : 2 * g + 2],
                pxv[:].rearrange("p (s c) -> p s c", s=2)[:, :, DOUT : DOUT + DA],
            )

        # ---- lr_col -> w = exp(lrc); the softmax max-shift cancels in
        # Num/denom and |lrc| is O(1), so no global max needed ----
        colp = colbuf[:]  # [P, nt, 2] contiguous
        c02 = small.tile([P, nt, DA], F32)
        nc.vector.tensor_scalar_mul(c02[:], colp, NEG_SLOPE)
        clr = small.tile([P, nt, DA], F32)
        nc.vector.tensor_max(clr[:], colp, c02[:])
        lr0 = small.tile([P, nt], F32)
        nc.vector.tensor_scalar(
            lr0[:], clr[:, :, 0], a2b[:, 0:1], None, mybir.AluOpType.mult
        )
        lr1 = small.tile([P, nt], F32)
        nc.vector.tensor_scalar(
            lr1[:], clr[:, :, 1], a2b[:, 1:2], None, mybir.AluOpType.mult
        )
        lrc = small.tile([P, nt], F32)
        nc.vector.tensor_add(lrc[:], lr0[:], lr1[:])
        w_all = consts.tile([P, nt], F32)
        nc.scalar.activation(w_all[:], lrc[:], mybir.ActivationFunctionType.Exp)

        # ---- U chunks [P, nt, UC] bf16: U[:,:,0:128]=w*xv0, U[:,:,128]=w ----
        # w column first so main MMs gate only on their own chunk's scale
        nc.scalar.copy(U[:, :, DOUT], w_all[:])
        for t in range(nt):
            src_v = pxvs[t // 2][:, (t % 2) * PW : (t % 2) * PW + DOUT]
            if t % 2 == 0:
                nc.vector.tensor_scalar(
                    U[:, t, 0:DOUT], src_v, w_all[:, t : t + 1], None,
                    mybir.AluOpType.mult,
                )
            else:
                nc.scalar.activation(
                    U[:, t, 0:DOUT], src_v,
                    mybir.ActivationFunctionType.Copy, scale=w_all[:, t : t + 1],
                )

        # ---- main loop over output row strips ----
        paccs = [None] * nt

        def mm(ti, tj):
            if paccs[ti] is None:
                paccs[ti] = ps.tile([P, UC], F32, tag="ring", name=f"pacc{ti}")
            nc.tensor.matmul(
                paccs[ti][:, 0 : DOUT + 1],
                mall[:, ti, tj * P : (tj + 1) * P],
                U[:, tj, 0 : DOUT + 1],
                start=(tj == 0),
                stop=(tj == nt - 1),
            )

        def finish(k):
            rec = small.tile([P, 1], F32, tag="rec")
            nc.vector.reciprocal(rec[:], paccs[k][:, DOUT : DOUT + 1])
            o1 = opool.tile([P, DOUT], F32, tag="o1")
            nc.vector.tensor_scalar(
                o1[:], paccs[k][:, 0:DOUT], rec[:], None, mybir.AluOpType.mult
            )
            o2 = opool.tile([P, DOUT], F32, tag="o2")
            nc.vector.tensor_add(o2[:], o1[:], bxb[:])
            # stores ride the SWDGE lanes so they never steal an HWDGE sem
            # lane from a mask load; the last two use HWDGE (lower latency on
            # the kernel tail, recycling lanes whose consumers are long done)
            if k >= nt - 2:
                nc.scalar.dma_start(out_d[k * P : (k + 1) * P, :], o2[:])
            else:
                nc.gpsimd.dma_start(out_d[k * P : (k + 1) * P, :], o2[:])

        for ti in range(nt):
            for tj in range(nt):
                mm(ti, tj)
            if ti >= skew:
                finish(ti - skew)
        for ti in range(max(0, nt - skew), nt):
            finish(ti)

    nc.compile()
    return nc


def host_inputs(xb_bf, L_b, wc, pk):
    """Per-core input map for batch b (weights replicated, host-prepped)."""
    return {
        "xT": np.ascontiguousarray(xb_bf.T),
        "maskt": L_b,
        "wcomb": wc,
        "pk": pk,
    }


_cached = {}


def _get_nc():
    if "nc" not in _cached:
        _cached["nc"] = build()
    return _cached["nc"]


def _install_ntff_shim():
    """The agent image's antenv lacks axon_hooks; synthesize it so
    run_bass_kernel_spmd(trace=True) can reach the .so's NTFF profiler."""
    import types

    try:
        import antenv.axon_hooks  # noqa: F401

        return True
    except ImportError:
        pass
    try:
        import antenv
        from trn_agent_boot.trn_boot import _ntff_profile_via_ctypes

        hook = _ntff_profile_via_ctypes("/opt/axon/libaxon_pjrt.so")
        mod = types.ModuleType("antenv.axon_hooks")
        _state = {"hook": hook}
        mod.set_axon_ntff_profile_hook = lambda h: _state.__setitem__("hook", h)
        mod.get_axon_ntff_profile_hook = lambda: _state["hook"]
        sys.modules["antenv.axon_hooks"] = mod
        antenv.axon_hooks = mod
        return hook is not None
    except Exception as e:
        print(f"ntff shim failed: {e}", file=sys.stderr)
        return False


def kernel(x, mask, Wr, Wc, Wcat, Wx, bx, _trace=False, **_unused):
    import ml_dtypes

    BF = ml_dtypes.bfloat16
    x = np.asarray(x)
    mask = np.asarray(mask)
    Wc = np.asarray(Wc)
    Wcat = np.asarray(Wcat)
    Wx = np.asarray(Wx)
    bx = np.asarray(bx)
    nc = _get_nc()
    if _trace:
        _trace = _install_ntff_shim()

    nt = N // P
    xb = x.astype(BF)  # [B, N, DIN]
    # tiled transpose: L[b, ti, p, tj, i] = mask[b, ti*128+i, tj*128+p]
    MD = ml_dtypes.float8_e4m3fn if MASK_FP8 else BF
    mb = mask.astype(np.float32).astype(MD)  # 0/1 exact in bf16/fp8
    L = np.ascontiguousarray(
        mb.reshape(B, nt, P, nt, P).transpose(0, 1, 4, 3, 2)
    ).reshape(B, N, N)
    wc = np.ascontiguousarray(np.concatenate([Wx.T, Wc.T], axis=1).astype(BF))
    pk1 = np.concatenate([Wcat[DA:].reshape(1, DA), bx.reshape(1, DOUT)], axis=1)
    pk = np.ascontiguousarray(
        np.broadcast_to(pk1, (P, DA + DOUT)), dtype=np.float32
    )
    in_maps = [host_inputs(xb[b], L[b], wc, pk) for b in range(B)]
    res = run_bass_kernel_spmd(nc, in_maps, core_ids=list(range(B)), trace=_trace)
    out = np.stack([res.results[c]["out"] for c in range(B)]).astype(np.float32)
    if _trace:
        kernel.last_results = res
    return out
